# revision 2
# baseline (speedup 1.0000x reference)
"""Trainium2 Bass kernel for nn_DN1_90864328114189 (topk_masking).

Computation (see reference): inpt = concat(x, y_response, z) (D=6208);
response = neurons @ inpt; three top-k paths (z: top-9 of 64, x: max of
2048, y: top-17 of 4096) produce `final`; 2047+16+8 rows of `neurons`
are rewritten as normalize(mix_g + s_r * inpt) and ages bump by 1.

Device strategy (8 cores, full I/O):
  Launch A  row-shard neurons [D,D] -> 8 slabs of [776, D].  Each core
            streams its slab once (19.3 MB): DVE scalar_tensor_tensor
            with accum_out gives the local gemv slice, and 343 PE
            matmuls (slab chunk as stationary, w as N=1 moving) give the
            partial x-mix  pmix = sum_r w_r * slab[r, :]  accumulated in
            PSUM [128, 49].
  host      tiny glue: merge per-core top-k candidates, compute the
            closed-form row norms (updated rows are rank-2 in
            {mix_g, inpt}), build per-core coefficient tables.
  Launch B  the 2071 updated rows are balanced 259/core; each 128-row
            tile is built on the PE as a K=4 matmul
            coeffs[4,128].T @ basis[4,512-chunk] and DMA'd out (6.4 MB
            per core).
Host assembles the full outputs (copy of neurons + row scatter).
"""

import numpy as np

import concourse.bass as bass
import concourse.mybir as mybir
from concourse.tile import TileContext
from concourse.bass_utils import run_bass_kernel_spmd

F32 = mybir.dt.float32

X_SIZE, NUM_NEURONS, Z_SIZE = 2048, 4096, 64
D = X_SIZE + NUM_NEURONS + Z_SIZE  # 6208
TOPK, ZTOPK = 16, 8
EPS_TIE = 1e-9
RND = 0.5

N_CORES = 8
RPC = D // N_CORES        # 776 rows per core
NT = 7                    # row tiles per core: 6 x 128 + 8
LAST_ROWS = RPC - 6 * 128  # 8
NCHUNK = 49               # 128-col chunks: 48 x 128 + 64
LAST_CHUNK = D - 48 * 128  # 64

N_UPD = (X_SIZE - 1) + TOPK + ZTOPK  # 2071 updated rows
B_ROWS = 259              # ceil(2071/8); core 7 gets one zero pad row
B_TILES = [(0, 128), (128, 128), (256, B_ROWS - 256)]
B_CHUNKS = [(i * 512, min(512, D - i * 512)) for i in range((D + 511) // 512)]

Y0 = X_SIZE
Z0 = X_SIZE + NUM_NEURONS


def _split_multi_waits(nc):
    """Walrus in this container rejects >1 sem-wait per instruction
    (setupSyncWait 'Too many sync wait commands').  Move extra waits onto
    standalone NoOps inserted just before, on the same engine."""
    for f in nc.m.functions:
        for bb in f.blocks:
            newlist = []
            for ins in bb.instructions:
                si = ins.sync_info
                if si is not None and len(si.on_wait) > 1:
                    waits = list(si.on_wait)
                    for w in waits[:-1]:
                        n = mybir.InstNoOp(
                            name=f"{ins.name}-w{w.id}", ins=[], outs=[]
                        )
                        n.engine = ins.engine
                        n.sync_info = mybir.SyncInfo(on_wait=[w], on_update=[])
                        newlist.append(n)
                    ins.sync_info = mybir.SyncInfo(
                        on_wait=[waits[-1]], on_update=list(si.on_update)
                    )
                newlist.append(ins)
            bb.instructions[:] = newlist


def build_launch_a():
    """Per-core gemv slice + partial x-mix over one [776, D] slab."""
    nc = bass.Bass()
    slab = nc.dram_tensor("slab", [RPC, D], F32, kind="ExternalInput")
    inpt_rep = nc.dram_tensor("inpt_rep", [128, D], F32, kind="ExternalInput")
    w_packed = nc.dram_tensor("w_packed", [128, NT], F32, kind="ExternalInput")
    resp_out = nc.dram_tensor("resp", [128, NT], F32, kind="ExternalOutput")
    pmix_out = nc.dram_tensor("pmix", [128, NCHUNK], F32, kind="ExternalOutput")

    with TileContext(nc) as tc:
        with (
            tc.tile_pool(name="cons", bufs=1) as cons,
            tc.tile_pool(name="slabp", bufs=3) as slabp,
            tc.tile_pool(name="trashp", bufs=2) as trashp,
            tc.tile_pool(name="psum", bufs=1, space="PSUM") as psump,
        ):
            irep = cons.tile([128, D], F32)
            nc.sync.dma_start(out=irep[:], in_=inpt_rep[:])
            wsb = cons.tile([128, NT], F32)
            nc.sync.dma_start(out=wsb[:], in_=w_packed[:])

            resp_sb = cons.tile([128, NT], F32)
            nc.vector.memset(resp_sb[:], 0.0)
            pm_psum = psump.tile([128, NCHUNK], F32)

            for t in range(NT):
                rows = 128 if t < NT - 1 else LAST_ROWS
                st = slabp.tile([128, D], F32, tag="slab")
                nc.sync.dma_start(
                    out=st[:rows, :], in_=slab[t * 128 : t * 128 + rows, :]
                )
                tr = trashp.tile([128, D], F32, tag="trash")
                nc.vector.scalar_tensor_tensor(
                    out=tr[:rows, :],
                    in0=st[:rows, :],
                    scalar=0.0,
                    in1=irep[:rows, :],
                    op0=mybir.AluOpType.bypass,
                    op1=mybir.AluOpType.mult,
                    accum_out=resp_sb[:rows, t : t + 1],
                )
                for c in range(NCHUNK):
                    cw = 128 if c < NCHUNK - 1 else LAST_CHUNK
                    nc.tensor.matmul(
                        pm_psum[:cw, c : c + 1],
                        st[:rows, c * 128 : c * 128 + cw],
                        wsb[:rows, t : t + 1],
                        start=(t == 0),
                        stop=(t == NT - 1),
                    )

            pm_sb = cons.tile([128, NCHUNK], F32)
            nc.vector.memset(pm_sb[:], 0.0)
            nc.vector.tensor_copy(pm_sb[:, : NCHUNK - 1], pm_psum[:, : NCHUNK - 1])
            nc.vector.tensor_copy(
                pm_sb[:LAST_CHUNK, NCHUNK - 1 :], pm_psum[:LAST_CHUNK, NCHUNK - 1 :]
            )
            nc.sync.dma_start(out=resp_out[:], in_=resp_sb[:])
            nc.sync.dma_start(out=pmix_out[:], in_=pm_sb[:])

    _split_multi_waits(nc)
    return nc


def build_launch_b():
    """Build 259 updated rows per core: rows = coeffs.T @ basis."""
    nc = bass.Bass()
    basis = nc.dram_tensor("basis", [4, D], F32, kind="ExternalInput")
    coeffs = nc.dram_tensor("coeffs", [4, B_ROWS], F32, kind="ExternalInput")
    rows_new = nc.dram_tensor("rows_new", [B_ROWS, D], F32, kind="ExternalOutput")

    with TileContext(nc) as tc:
        with (
            tc.tile_pool(name="cons", bufs=1) as cons,
            tc.tile_pool(name="outp", bufs=2) as outp,
            tc.tile_pool(name="psum", bufs=4, space="PSUM") as psump,
        ):
            bas = cons.tile([4, D], F32)
            nc.sync.dma_start(out=bas[:], in_=basis[:])
            cf = cons.tile([4, B_ROWS], F32)
            nc.sync.dma_start(out=cf[:], in_=coeffs[:])

            for r0, rk in B_TILES:
                ot = outp.tile([128, D], F32, tag="o")
                for c0, cw in B_CHUNKS:
                    ps = psump.tile([128, 512], F32, tag="ps")
                    nc.tensor.matmul(
                        ps[:rk, :cw],
                        cf[:, r0 : r0 + rk],
                        bas[:, c0 : c0 + cw],
                        start=True,
                        stop=True,
                    )
                    nc.vector.tensor_copy(ot[:rk, c0 : c0 + cw], ps[:rk, :cw])
                nc.sync.dma_start(out=rows_new[r0 : r0 + rk, :], in_=ot[:rk, :])

    _split_multi_waits(nc)
    return nc


_NC_A = None
_NC_B = None


def _get_kernels():
    global _NC_A, _NC_B
    if _NC_A is None:
        _NC_A = build_launch_a()
        _NC_B = build_launch_b()
    return _NC_A, _NC_B


def _unpack_cols(packed, n):
    """[128, T] with column t holding rows t*128..t*128+127 -> flat [n]."""
    return np.ascontiguousarray(packed.T).reshape(-1)[:n]


def _topk_desc(vals, k):
    """lax.top_k semantics: sorted descending, ties -> lower index."""
    idx = np.argsort(-vals, kind="stable")[:k]
    return vals[idx], idx


def kernel(x, z, y_response, neurons, ages, _profile=False):
    x = np.asarray(x, np.float32)
    z = np.asarray(z, np.float32)
    y_response = np.asarray(y_response, np.float32)
    neurons = np.ascontiguousarray(np.asarray(neurons, np.float32))
    ages = np.asarray(ages, np.float32)

    nc_a, nc_b = _get_kernels()

    inpt = np.concatenate([x, y_response, z]).astype(np.float32)
    inpt_rep = np.ascontiguousarray(np.broadcast_to(inpt, (128, D)))

    # x-mix weights: rows 0..2046 get (a-1)/a, everything else 0.
    w_global = np.zeros(D, np.float64)
    ax = ages[: X_SIZE - 1].astype(np.float64)
    w_global[: X_SIZE - 1] = (ax - 1.0) / ax
    w_global = w_global.astype(np.float32)

    in_maps_a = []
    for c in range(N_CORES):
        wc = np.zeros(NT * 128, np.float32)
        wc[:RPC] = w_global[c * RPC : (c + 1) * RPC]
        w_packed = np.ascontiguousarray(wc.reshape(NT, 128).T)
        in_maps_a.append(
            {
                "slab": neurons[c * RPC : (c + 1) * RPC],
                "inpt_rep": inpt_rep,
                "w_packed": w_packed,
            }
        )

    res_a = run_bass_kernel_spmd(
        nc_a, in_maps_a, list(range(N_CORES)), trace=_profile
    )

    response = np.concatenate(
        [_unpack_cols(res_a.results[c]["resp"], RPC) for c in range(N_CORES)]
    )
    pmix = np.zeros(D, np.float64)
    for c in range(N_CORES):
        pmix += _unpack_cols(res_a.results[c]["pmix"], D).astype(np.float64)
    mix_x = pmix.astype(np.float32)

    # ---- host glue: top-k, final vector, row coefficients ----
    rx = response[:X_SIZE]
    ry = response[Y0:Z0]
    rz = response[Z0:]

    zvals, zidx = _topk_desc(rz, ZTOPK + 1)
    gz = Z0 + zidx[:-1]
    yvals, yidx = _topk_desc(ry, TOPK + 1)
    gy = Y0 + yidx[:-1]

    final = np.zeros(D, np.float32)
    final[gz] = ((zvals[:-1] - zvals[-1]) / (zvals[0] - zvals[-1])).astype(
        np.float32
    )
    tx = 1.0 if np.all(rx == 0.0) else 0.0
    xvals = (rx / (np.max(rx) + EPS_TIE * tx * RND)).astype(np.float32)
    final[:X_SIZE] = xvals
    ty = 1.0 if np.any(yvals[:-1] == yvals[-1]) else 0.0
    final[gy] = (
        (yvals[:-1] - yvals[-1]) / (yvals[0] - yvals[-1] + EPS_TIE * ty * RND)
    ).astype(np.float32)

    # mixes for y/z paths from the INPUT rows (host: <= 16 x D dots).
    def _mix(rows_idx):
        a = ages[rows_idx].astype(np.float64)
        wts = (a - 1.0) / a
        return (wts[None, :] @ neurons[rows_idx].astype(np.float64))[0]

    mix_y64 = _mix(gy)
    mix_z64 = _mix(gz)
    mix_y = mix_y64.astype(np.float32)
    mix_z = mix_z64.astype(np.float32)

    basis = np.ascontiguousarray(np.stack([mix_x, mix_y, mix_z, inpt]))

    # closed-form norms: ||mix + s*inpt||^2 = ||mix||^2 + 2s<mix,inpt> + s^2||inpt||^2
    inpt64 = inpt.astype(np.float64)
    ii = float(inpt64 @ inpt64)
    stats = {}
    for g, m in enumerate([mix_x.astype(np.float64), mix_y64, mix_z64]):
        stats[g] = (float(m @ m), float(m @ inpt64))

    upd_rows = np.concatenate([np.arange(X_SIZE - 1), gy, gz])
    upd_group = np.concatenate(
        [
            np.zeros(X_SIZE - 1, np.int64),
            np.ones(TOPK, np.int64),
            np.full(ZTOPK, 2, np.int64),
        ]
    )
    upd_vals = np.concatenate(
        [xvals[: X_SIZE - 1].astype(np.float64), yvals[:-1], zvals[:-1]]
    )
    a_upd = ages[upd_rows].astype(np.float64)
    s_upd = upd_vals / a_upd
    mm = np.array([stats[g][0] for g in range(3)])[upd_group]
    mi = np.array([stats[g][1] for g in range(3)])[upd_group]
    n_upd = np.sqrt(mm + 2.0 * s_upd * mi + s_upd * s_upd * ii) + 1e-12
    alpha = (1.0 / n_upd).astype(np.float32)
    beta = (s_upd / n_upd).astype(np.float32)

    in_maps_b = []
    for c in range(N_CORES):
        lo = c * B_ROWS
        hi = min(lo + B_ROWS, N_UPD)
        cnt = hi - lo
        c4 = np.zeros((4, B_ROWS), np.float32)
        if cnt > 0:
            gsel = upd_group[lo:hi]
            c4[gsel, np.arange(cnt)] = alpha[lo:hi]
            c4[3, :cnt] = beta[lo:hi]
        in_maps_b.append({"basis": basis, "coeffs": c4})

    res_b = run_bass_kernel_spmd(
        nc_b, in_maps_b, list(range(N_CORES)), trace=_profile
    )

    new_rows = np.concatenate(
        [res_b.results[c]["rows_new"] for c in range(N_CORES)]
    )[:N_UPD]

    neurons_out = neurons.copy()
    neurons_out[upd_rows] = new_rows
    ages_out = ages.copy()
    ages_out[upd_rows] += 1.0

    z_response = final[Z0:].copy()

    if _profile:
        t_a = res_a.exec_time_ns or 0
        t_b = res_b.exec_time_ns or 0
        kernel.last_exec_time_ns = t_a + t_b
        kernel.last_exec_split = (t_a, t_b)

    return z_response, final, neurons_out, ages_out
